# revision 18
# baseline (speedup 1.0000x reference)
"""Trainium2 Bass kernel for nn_EncodingNetwork (gnn_message_passing).

Math (exact collapse of the reference):
    enc       = x @ W_enc.T + b_enc                    [N=200, D=1024]
    cm[w]     = class-mean of enc = xm[w] @ W_enc.T + b_enc   (xm = class-mean of x)
    gm        = mean(enc, axis=0) = mean(cm, axis=0)
    per_class = cm @ Wl.T + gm @ Wr.T + b_rel          [20, 2D]
    out       = gaussian * per_class[:, D:] + per_class[:, :D]

Sharding across 8 cores: the final 1024 output columns are split 128/core.
Each core loads only its 128-row slices of W_rel (2 MB instead of 16 MB) and
a 1024x128 column slice of W_enc (0.5 MB).  The class-mean matrix cm is
computed as a per-core 128-column slice and assembled with an 8-core
AllGather.  Everything on device runs in transposed [feature, class] layout
so the contraction dim always sits on SBUF partitions.
"""

import numpy as np

import concourse.bass as bass
import concourse.tile as tile
from concourse import bacc

from concourse import mybir
from concourse.bass import ts
from concourse.bass_utils import run_bass_kernel_spmd

N_WAY = 20
N_SUPPORT = 10
N = N_WAY * N_SUPPORT  # 200
D = 1024
NC = 8
SL = D // NC  # 128 output columns per core
KT = D // 128  # 8 contraction tiles
F32 = mybir.dt.float32

USE_ALLGATHER = True


def _build_nc(use_ag: bool) -> bacc.Bacc:
    nc = bacc.Bacc(
        "TRN2", target_bir_lowering=False, debug=False, num_devices=NC
    )

    # wblob free-dim column layout per contraction tile kt:
    #   [0:wenc_w)           W_enc^T chunk (wenc_w = SL for AG, D for replicated)
    #   [wenc_w + 0:128)     A = Wl^T chunk for the means rows   (W_rel[S_c, :D])
    #   [wenc_w + 128:256)   B = Wr^T chunk for the means rows   (W_rel[S_c, D:])
    #   [wenc_w + 256:384)   C = Wl^T chunk for the stds rows    (W_rel[D+S_c, :D])
    #   [wenc_w + 384:512)   D = Wr^T chunk for the stds rows    (W_rel[D+S_c, D:])
    wenc_w = SL if use_ag else D
    wcols = wenc_w + 4 * 128

    XW = D + N_WAY + 31
    bw = 2 * XW + KT * wcols
    blob_h = nc.declare_dram_parameter("blob", [128, bw], F32, isOutput=False)
    out_h = nc.declare_dram_parameter("out", [128, N_WAY], F32, isOutput=True)

    if use_ag:
        cc_in = nc.dram_tensor("cc_in", [128, N_WAY], F32)
        cc_out = nc.dram_tensor("cc_out", [D, N_WAY], F32, addr_space="Shared")

    with tile.TileContext(nc) as tc:
        with (
            tc.tile_pool(name="sbuf", bufs=1) as sb,
            tc.tile_pool(name="psum1", bufs=1, space="PSUM") as ps,
            tc.tile_pool(name="psum2", bufs=2, space="PSUM") as ps2,
        ):
            # ---- load: ONE host-packed blob = x|selector|smalls|weights.
            # Walrus rejects >1 semaphore wait on normal instructions and
            # ~>7 on the kernel-tail Drain, which waits once per DMA lane;
            # a single input DMA keeps the whole kernel at 4 DMAs.
            blob = sb.tile([128, bw], F32, tag="blob")
            nc.gpsimd.dma_start(blob[:], blob_h[:])
            # DVE-owned staging copy: later vector ops then only ever depend
            # on same-engine data, keeping each one at <=1 semaphore wait.
            smw = sb.tile([128, 31], F32, tag="smw")
            nc.vector.tensor_copy(smw[:], blob[:, D + N_WAY : XW])
            woff = 2 * XW

            # ---- stage 1: xm^T [d, w] = x^T @ S, built 128-row chunk at a time
            xm_sb = sb.tile([128, KT * N_WAY], F32, tag="xm")
            for t in range(KT):
                p = ps2.tile([128, N_WAY], F32, tag="xm_ps")
                for i in range(2):
                    nc.tensor.matmul(
                        p[:],
                        blob[:, i * XW + t * 128 : i * XW + (t + 1) * 128],
                        blob[:, i * XW + D : i * XW + D + N_WAY],
                        start=(i == 0),
                        stop=(i == 1),
                    )
                nc.vector.tensor_copy(xm_sb[:, ts(t, N_WAY)], p[:])

            # ---- stage 2: cm^T = W_enc^T-chunks @ xm^T (+ b_enc)
            cmf_sb = sb.tile([128, KT * N_WAY], F32, tag="cmf")
            if use_ag:
                pcm = ps.tile([128, N_WAY], F32, tag="cm_ps")
                for kt in range(KT):
                    nc.tensor.matmul(
                        pcm[:],
                        blob[:, woff + kt * wcols : woff + kt * wcols + SL],
                        xm_sb[:, ts(kt, N_WAY)],
                        start=(kt == 0),
                        stop=(kt == KT - 1),
                    )
                cm_own = sb.tile([128, N_WAY], F32, tag="cm_own")
                nc.vector.tensor_copy(cm_own[:], pcm[:])
                nc.vector.tensor_add(
                    cm_own[:], cm_own[:], smw[:, 30:31].broadcast_to((128, N_WAY))
                )
                # ---- stage 3: AllGather the 8 column slices of cm^T
                nc.gpsimd.dma_start(cc_in[:], cm_own[:])
                nc.gpsimd.collective_compute(
                    "AllGather",
                    mybir.AluOpType.bypass,
                    replica_groups=[list(range(NC))],
                    ins=[cc_in[:]],
                    outs=[cc_out[:]],
                )
                nc.gpsimd.dma_start(
                    cmf_sb[:].rearrange("p (t w) -> p t w", t=KT),
                    cc_out[:].rearrange("(t p) w -> p t w", p=128),
                )
            else:
                for t in range(KT):
                    pcm = ps2.tile([128, N_WAY], F32, tag="cm_ps")
                    for kt in range(KT):
                        nc.tensor.matmul(
                            pcm[:],
                            blob[:, woff + kt * wcols + t * 128 : woff + kt * wcols + (t + 1) * 128],
                            xm_sb[:, ts(kt, N_WAY)],
                            start=(kt == 0),
                            stop=(kt == KT - 1),
                        )
                    nc.vector.tensor_copy(cmf_sb[:, ts(t, N_WAY)], pcm[:])
                    nc.vector.tensor_add(
                        cmf_sb[:, ts(t, N_WAY)], cmf_sb[:, ts(t, N_WAY)],
                        smw[:, t : t + 1].broadcast_to((128, N_WAY)),
                    )

            # ---- stage 4: the four rel products against cm^T
            pm = ps.tile([128, N_WAY], F32, tag="pm")
            pmR = ps.tile([128, N_WAY], F32, tag="pmR")
            pstd = ps.tile([128, N_WAY], F32, tag="pstd")
            pstdR = ps.tile([128, N_WAY], F32, tag="pstdR")
            for kt in range(KT):
                rhs = cmf_sb[:, ts(kt, N_WAY)]
                st, sp = (kt == 0), (kt == KT - 1)
                o = woff + kt * wcols + wenc_w
                nc.tensor.matmul(pm[:], blob[:, o : o + 128], rhs, start=st, stop=sp)
                nc.tensor.matmul(pmR[:], blob[:, o + 128 : o + 256], rhs, start=st, stop=sp)
                nc.tensor.matmul(pstd[:], blob[:, o + 256 : o + 384], rhs, start=st, stop=sp)
                nc.tensor.matmul(pstdR[:], blob[:, o + 384 : o + 512], rhs, start=st, stop=sp)

            # ---- stage 5: fold the rhs-term row-means + biases, combine
            rm = sb.tile([128, 1], F32, tag="rm")
            rs = sb.tile([128, 1], F32, tag="rs")
            nc.vector.reduce_sum(rm[:], pmR[:], axis=mybir.AxisListType.X)
            nc.vector.reduce_sum(rs[:], pstdR[:], axis=mybir.AxisListType.X)
            nc.vector.tensor_scalar_mul(rm[:], rm[:], 1.0 / N_WAY)
            nc.vector.tensor_scalar_mul(rs[:], rs[:], 1.0 / N_WAY)
            bias_m = sb.tile([128, 1], F32, tag="bias_m")
            bias_s = sb.tile([128, 1], F32, tag="bias_s")
            nc.vector.tensor_add(bias_m[:], rm[:], smw[:, 8:9])
            nc.vector.tensor_add(bias_s[:], rs[:], smw[:, 9:10])
            t_s = sb.tile([128, N_WAY], F32, tag="t_s")
            nc.vector.tensor_add(
                t_s[:], pstd[:], bias_s[:].broadcast_to((128, N_WAY))
            )
            t_sg = sb.tile([128, N_WAY], F32, tag="t_sg")
            nc.vector.tensor_mul(t_sg[:], t_s[:], smw[:, 10:30])
            t_m = sb.tile([128, N_WAY], F32, tag="t_m")
            nc.vector.tensor_add(
                t_m[:], pm[:], bias_m[:].broadcast_to((128, N_WAY))
            )
            out_sb = sb.tile([128, N_WAY], F32, tag="out")
            nc.vector.tensor_add(out_sb[:], t_sg[:], t_m[:])
            nc.gpsimd.dma_start(out_h[:], out_sb[:])

    nc.finalize()
    return nc


_NC_CACHE: dict[bool, bacc.Bacc] = {}


def _get_nc(use_ag: bool) -> bacc.Bacc:
    if use_ag not in _NC_CACHE:
        _NC_CACHE[use_ag] = _build_nc(use_ag)
    return _NC_CACHE[use_ag]


def _make_in_maps(x, W_enc, b_enc, W_rel, b_rel, gaussian, use_ag):
    # xs: [2, 128, D+20+31] — x rows (zero-padded to 256) | class-selector/10
    # | 31 cols of per-core "smalls" (biases + gaussian slice, tile 0 only)
    xs = np.zeros((2, 128, D + N_WAY + 31), np.float32)
    xs[:, :, :D].reshape(256, D)[:N] = x
    sel = np.zeros((N, N_WAY), np.float32)
    sel[np.arange(N), np.arange(N) // N_SUPPORT] = 1.0 / N_SUPPORT
    xs[:, :, D : D + N_WAY].reshape(256, N_WAY)[:N] = sel

    wenc_w = SL if use_ag else D
    in_maps = []
    for c in range(NC):
        s = slice(c * SL, (c + 1) * SL)
        s2 = slice(D + c * SL, D + (c + 1) * SL)
        blk = np.empty((KT, 128, wenc_w + 4 * 128), np.float32)
        if use_ag:
            # W_enc^T column slice: [D, SL] -> chunks of 128 rows
            blk[:, :, :SL] = np.ascontiguousarray(W_enc[s, :].T).reshape(KT, 128, SL)
        else:
            blk[:, :, :D] = np.ascontiguousarray(W_enc.T).reshape(KT, 128, D)
        o = wenc_w
        for i, m in enumerate(
            (W_rel[s, :D], W_rel[s, D:], W_rel[s2, :D], W_rel[s2, D:])
        ):
            blk[:, :, o + i * 128 : o + (i + 1) * 128] = (
                np.ascontiguousarray(m.T).reshape(KT, 128, SL)
            )
        xsc = xs.copy()
        sm = xsc[0, :, D + N_WAY :]
        sm[:, 0:8] = b_enc.reshape(KT, 128).T
        sm[:, 8] = b_rel[s]
        sm[:, 9] = b_rel[s2]
        sm[:, 10:30] = gaussian[:, s].T
        sm[:, 30] = b_enc[s]
        blob = np.concatenate(
            [xsc[0], xsc[1], blk.transpose(1, 0, 2).reshape(128, -1)], axis=1
        )
        in_maps.append({"blob": np.ascontiguousarray(blob)})
    return in_maps


def run(inputs: dict, trace: bool = False, use_ag: bool = USE_ALLGATHER):
    x = np.asarray(inputs["x_support"], np.float32)
    W_enc = np.asarray(inputs["W_enc"], np.float32)
    b_enc = np.asarray(inputs["b_enc"], np.float32)
    W_rel = np.asarray(inputs["W_rel"], np.float32)
    b_rel = np.asarray(inputs["b_rel"], np.float32)
    gaussian = np.asarray(inputs["gaussian_vectors"], np.float32)

    nc = _get_nc(use_ag)
    in_maps = _make_in_maps(x, W_enc, b_enc, W_rel, b_rel, gaussian, use_ag)
    res = run_bass_kernel_spmd(nc, in_maps, list(range(NC)), trace=trace)

    out = np.empty((N_WAY, D), np.float32)
    for c in range(NC):
        out[:, c * SL : (c + 1) * SL] = res.results[c]["out"].T
    return out, res


def kernel(**inputs) -> np.ndarray:
    out, _ = run(inputs)
    return out


# revision 19
# speedup vs baseline: 1.2809x; 1.2809x over previous
"""Trainium2 Bass kernel for nn_EncodingNetwork (gnn_message_passing).

Math (exact collapse of the reference):
    enc       = x @ W_enc.T + b_enc                    [N=200, D=1024]
    cm[w]     = class-mean of enc = xm[w] @ W_enc.T + b_enc   (xm = class-mean of x)
    gm        = mean(enc, axis=0) = mean(cm, axis=0)
    per_class = cm @ Wl.T + gm @ Wr.T + b_rel          [20, 2D]
    out       = gaussian * per_class[:, D:] + per_class[:, :D]

Sharding across 8 cores: the final 1024 output columns are split 128/core.
Each core loads only its 128-row slices of W_rel (2 MB instead of 16 MB) and
a 1024x128 column slice of W_enc (0.5 MB).  The class-mean matrix cm is
computed as a per-core 128-column slice and assembled with an 8-core
AllGather.  Everything on device runs in transposed [feature, class] layout
so the contraction dim always sits on SBUF partitions.

Implementation notes (hard-won on this toolchain):
  - bacc.Bacc + nc.finalize() are required: raw bass.Bass programs emit
    multi-semaphore waits that walrus codegen rejects ("Too many sync wait
    commands"); Bacc legalizes them.
  - Big loads must go through the HW-DGE rings (nc.sync / nc.scalar):
    SWDGE (gpsimd) generates descriptors in ucode at ~1us each, which
    serializes a [128, N] load into ~100 us of trickle.
  - Inputs are host-packed into three blobs so the pre-AllGather path
    (x + selector + smalls + W_enc slice) lands early while the 2 MB
    W_rel slice streams in parallel.
"""

import numpy as np

import concourse.bass as bass  # noqa: F401
import concourse.tile as tile
from concourse import bacc, mybir
from concourse.bass import ts
from concourse.bass_utils import run_bass_kernel_spmd

N_WAY = 20
N_SUPPORT = 10
N = N_WAY * N_SUPPORT  # 200
D = 1024
NC = 8
SL = D // NC  # 128 output columns per core
KT = D // 128  # 8 contraction tiles
XW = D + N_WAY + 31  # x | selector | smalls columns, per 128-row tile
F32 = mybir.dt.float32

USE_ALLGATHER = True


def _build_nc(use_ag: bool) -> bacc.Bacc:
    nc = bacc.Bacc("TRN2", target_bir_lowering=False, debug=False, num_devices=NC)

    # wcm per contraction tile kt: the W_enc^T chunk ([128, SL] for the AG
    # variant, [128, D] replicated otherwise).
    # wrel per contraction tile kt, four 128-wide blocks:
    #   A = Wl^T chunk for the means rows   (W_rel[S_c, :D])
    #   B = Wr^T chunk for the means rows   (W_rel[S_c, D:])
    #   C = Wl^T chunk for the stds rows    (W_rel[D+S_c, :D])
    #   Dd= Wr^T chunk for the stds rows    (W_rel[D+S_c, D:])
    wenc_w = SL if use_ag else D

    xs_h = nc.declare_dram_parameter("xsb", [128, 2 * XW], F32, isOutput=False)
    wcm_h = nc.declare_dram_parameter("wcm", [128, KT * wenc_w], F32, isOutput=False)
    wrel_h = nc.declare_dram_parameter("wrel", [128, KT * 512], F32, isOutput=False)
    out_h = nc.declare_dram_parameter("out", [128, N_WAY], F32, isOutput=True)

    if use_ag:
        cc_in = nc.dram_tensor("cc_in", [128, N_WAY], F32)
        cc_out = nc.dram_tensor("cc_out", [D, N_WAY], F32, addr_space="Shared")

    with tile.TileContext(nc) as tc:
        with (
            tc.tile_pool(name="sbuf", bufs=1) as sb,
            tc.tile_pool(name="psum1", bufs=1, space="PSUM") as ps,
            tc.tile_pool(name="psum2", bufs=2, space="PSUM") as ps2,
        ):
            # ---- loads: HW-DGE rings; Act ring gets the early x blob ----
            xs_all = sb.tile([128, 2 * XW], F32, tag="xs")
            nc.scalar.dma_start(xs_all[:], xs_h[:])
            wcm_all = sb.tile([128, KT * wenc_w], F32, tag="wcm")
            nc.scalar.dma_start(wcm_all[:], wcm_h[:])
            wrel_all = sb.tile([128, KT * 512], F32, tag="wrel")
            nc.sync.dma_start(wrel_all[:], wrel_h[:])

            smw = sb.tile([128, 31], F32, tag="smw")
            nc.vector.tensor_copy(smw[:], xs_all[:, D + N_WAY : XW])

            # ---- stage 1: xm^T [d, w] = x^T @ S, built 128-row chunk at a time
            xm_sb = sb.tile([128, KT * N_WAY], F32, tag="xm")
            for t in range(KT):
                p = ps2.tile([128, N_WAY], F32, tag="xm_ps")
                for i in range(2):
                    nc.tensor.matmul(
                        p[:],
                        xs_all[:, i * XW + t * 128 : i * XW + (t + 1) * 128],
                        xs_all[:, i * XW + D : i * XW + D + N_WAY],
                        start=(i == 0),
                        stop=(i == 1),
                    )
                nc.vector.tensor_copy(xm_sb[:, ts(t, N_WAY)], p[:])

            # ---- stage 2: cm^T = W_enc^T-chunks @ xm^T (+ b_enc)
            cmf_sb = sb.tile([128, KT * N_WAY], F32, tag="cmf")
            if use_ag:
                pcm = ps.tile([128, N_WAY], F32, tag="cm_ps")
                for kt in range(KT):
                    nc.tensor.matmul(
                        pcm[:],
                        wcm_all[:, ts(kt, SL)],
                        xm_sb[:, ts(kt, N_WAY)],
                        start=(kt == 0),
                        stop=(kt == KT - 1),
                    )
                cm_own = sb.tile([128, N_WAY], F32, tag="cm_own")
                nc.vector.tensor_copy(cm_own[:], pcm[:])
                nc.vector.tensor_add(
                    cm_own[:], cm_own[:], smw[:, 30:31].broadcast_to((128, N_WAY))
                )
                # ---- stage 3: AllGather the 8 column slices of cm^T
                nc.sync.dma_start(cc_in[:], cm_own[:])
                nc.gpsimd.collective_compute(
                    "AllGather",
                    mybir.AluOpType.bypass,
                    replica_groups=[list(range(NC))],
                    ins=[cc_in[:]],
                    outs=[cc_out[:]],
                )
                nc.sync.dma_start(
                    cmf_sb[:].rearrange("p (t w) -> p t w", t=KT),
                    cc_out[:].rearrange("(t p) w -> p t w", p=128),
                )
            else:
                for t in range(KT):
                    pcm = ps2.tile([128, N_WAY], F32, tag="cm_ps")
                    for kt in range(KT):
                        nc.tensor.matmul(
                            pcm[:],
                            wcm_all[:, kt * D + t * 128 : kt * D + (t + 1) * 128],
                            xm_sb[:, ts(kt, N_WAY)],
                            start=(kt == 0),
                            stop=(kt == KT - 1),
                        )
                    nc.vector.tensor_copy(cmf_sb[:, ts(t, N_WAY)], pcm[:])
                    nc.vector.tensor_add(
                        cmf_sb[:, ts(t, N_WAY)],
                        cmf_sb[:, ts(t, N_WAY)],
                        smw[:, t : t + 1].broadcast_to((128, N_WAY)),
                    )

            # ---- stage 4: the four rel products against cm^T
            pm = ps.tile([128, N_WAY], F32, tag="pm")
            pmR = ps.tile([128, N_WAY], F32, tag="pmR")
            pstd = ps.tile([128, N_WAY], F32, tag="pstd")
            pstdR = ps.tile([128, N_WAY], F32, tag="pstdR")
            for kt in range(KT):
                rhs = cmf_sb[:, ts(kt, N_WAY)]
                st, sp = (kt == 0), (kt == KT - 1)
                o = kt * 512
                nc.tensor.matmul(pm[:], wrel_all[:, o : o + 128], rhs, start=st, stop=sp)
                nc.tensor.matmul(pmR[:], wrel_all[:, o + 128 : o + 256], rhs, start=st, stop=sp)
                nc.tensor.matmul(pstd[:], wrel_all[:, o + 256 : o + 384], rhs, start=st, stop=sp)
                nc.tensor.matmul(pstdR[:], wrel_all[:, o + 384 : o + 512], rhs, start=st, stop=sp)

            # ---- stage 5: fold the rhs-term row-means + biases, combine
            rm = sb.tile([128, 1], F32, tag="rm")
            rs = sb.tile([128, 1], F32, tag="rs")
            nc.vector.reduce_sum(rm[:], pmR[:], axis=mybir.AxisListType.X)
            nc.vector.reduce_sum(rs[:], pstdR[:], axis=mybir.AxisListType.X)
            nc.vector.tensor_scalar_mul(rm[:], rm[:], 1.0 / N_WAY)
            nc.vector.tensor_scalar_mul(rs[:], rs[:], 1.0 / N_WAY)
            bias_m = sb.tile([128, 1], F32, tag="bias_m")
            bias_s = sb.tile([128, 1], F32, tag="bias_s")
            nc.vector.tensor_add(bias_m[:], rm[:], smw[:, 8:9])
            nc.vector.tensor_add(bias_s[:], rs[:], smw[:, 9:10])
            t_s = sb.tile([128, N_WAY], F32, tag="t_s")
            nc.vector.tensor_add(
                t_s[:], pstd[:], bias_s[:].broadcast_to((128, N_WAY))
            )
            t_sg = sb.tile([128, N_WAY], F32, tag="t_sg")
            nc.vector.tensor_mul(t_sg[:], t_s[:], smw[:, 10:30])
            t_m = sb.tile([128, N_WAY], F32, tag="t_m")
            nc.vector.tensor_add(
                t_m[:], pm[:], bias_m[:].broadcast_to((128, N_WAY))
            )
            out_sb = sb.tile([128, N_WAY], F32, tag="out")
            nc.vector.tensor_add(out_sb[:], t_sg[:], t_m[:])
            nc.sync.dma_start(out_h[:], out_sb[:])

    nc.finalize()
    return nc


_NC_CACHE: dict[bool, bacc.Bacc] = {}


def _get_nc(use_ag: bool) -> bacc.Bacc:
    if use_ag not in _NC_CACHE:
        _NC_CACHE[use_ag] = _build_nc(use_ag)
    return _NC_CACHE[use_ag]


def _make_in_maps(x, W_enc, b_enc, W_rel, b_rel, gaussian, use_ag):
    # xsb: [128, 2*XW] — two 128-row tiles of [x | selector/10 | smalls]
    xs = np.zeros((2, 128, XW), np.float32)
    xs[:, :, :D].reshape(256, D)[:N] = x
    sel = np.zeros((N, N_WAY), np.float32)
    sel[np.arange(N), np.arange(N) // N_SUPPORT] = 1.0 / N_SUPPORT
    xs[:, :, D : D + N_WAY].reshape(256, N_WAY)[:N] = sel

    in_maps = []
    for c in range(NC):
        s = slice(c * SL, (c + 1) * SL)
        s2 = slice(D + c * SL, D + (c + 1) * SL)
        if use_ag:
            # W_enc^T column slice: [D, SL] -> [128, KT*SL] chunk-interleaved
            wcm = (
                np.ascontiguousarray(W_enc[s, :].T)
                .reshape(KT, 128, SL)
                .transpose(1, 0, 2)
                .reshape(128, KT * SL)
            )
        else:
            wcm = (
                np.ascontiguousarray(W_enc.T)
                .reshape(KT, 128, D)
                .transpose(1, 0, 2)
                .reshape(128, KT * D)
            )
        blk = np.empty((KT, 128, 512), np.float32)
        for i, m in enumerate(
            (W_rel[s, :D], W_rel[s, D:], W_rel[s2, :D], W_rel[s2, D:])
        ):
            blk[:, :, i * 128 : (i + 1) * 128] = (
                np.ascontiguousarray(m.T).reshape(KT, 128, SL)
            )
        wrel = blk.transpose(1, 0, 2).reshape(128, KT * 512)

        xsc = xs.copy()
        sm = xsc[0, :, D + N_WAY :]
        sm[:, 0:8] = b_enc.reshape(KT, 128).T
        sm[:, 8] = b_rel[s]
        sm[:, 9] = b_rel[s2]
        sm[:, 10:30] = gaussian[:, s].T
        sm[:, 30] = b_enc[s]
        in_maps.append(
            {
                "xsb": np.ascontiguousarray(xsc.transpose(1, 0, 2).reshape(128, -1)),
                "wcm": np.ascontiguousarray(wcm),
                "wrel": np.ascontiguousarray(wrel),
            }
        )
    return in_maps


def run(inputs: dict, trace: bool = False, use_ag: bool = USE_ALLGATHER):
    x = np.asarray(inputs["x_support"], np.float32)
    W_enc = np.asarray(inputs["W_enc"], np.float32)
    b_enc = np.asarray(inputs["b_enc"], np.float32)
    W_rel = np.asarray(inputs["W_rel"], np.float32)
    b_rel = np.asarray(inputs["b_rel"], np.float32)
    gaussian = np.asarray(inputs["gaussian_vectors"], np.float32)

    nc = _get_nc(use_ag)
    in_maps = _make_in_maps(x, W_enc, b_enc, W_rel, b_rel, gaussian, use_ag)
    res = run_bass_kernel_spmd(nc, in_maps, list(range(NC)), trace=trace)

    out = np.empty((N_WAY, D), np.float32)
    for c in range(NC):
        out[:, c * SL : (c + 1) * SL] = res.results[c]["out"].T
    return out, res


def kernel(**inputs) -> np.ndarray:
    out, _ = run(inputs)
    return out
